# revision 4
# baseline (speedup 1.0000x reference)
"""Trainium2 Bass kernel for nn_LogLinearAttention.

Math: the reference computes
    q = x@Wq.T+bq ; v = x@Wv.T+bv ; r = x@Wr.T+br
    scores = q @ v.T ; attn = softmax(scores, axis=1)   # over the QUERY axis
    emb[b,s,:] = sum_t attn[b,s,t] r[b,t,:] ; pooled = emb.sum(axis=1)
    out = sigmoid(pooled @ Wl.T + bl)

Because softmax normalizes over axis 1 and pooled sums over that same
axis, sum_s attn[s, t] == 1 for every t, so
    pooled[b] = sum_t r[b, t, :] = (sum_t x[b, t, :]) @ Wr.T + S*br
and the q/v projections and the S x S attention cancel exactly:
    out[b] = sigmoid( xsum[b] . (Wl@Wr) + S*(br . Wl) + bl )

The kernel therefore only needs a sequence-sum of x (the only large
input, 32MB total) plus tiny weight contractions. Data-parallel over
batch: core b handles x[b] (4MB), weights replicated.

Per-core device program (v5 — no collective):
  v4 sharded the Wl@Wr contraction over cores and AllGathered the
  shards; the trace showed that 32KB AllGather costing ~65us of
  fixed latency (44us barrier + 54us cc trigger delay) while the x
  stream was done by 28us. v5 instead replicates the full Wr (1MB
  extra DMA per core, ~4us at ring bandwidth) and computes everything
  core-locally:
  - weights pack into ONE [128,2057] DMA, first on the scalar ring:
    per j in 0..3 a [Wr rows 4p+j | br col] block of width 513, then
    Wl as [128,4] and bl.
  - x[b] arrives as 16 slice DMAs of [128,512] (256KB each), 10 on
    the sync HWDGE ring, 6 after the weights on the scalar ring
    (only SP+Activation have HWDGE; the two rings together run at
    ~330GB/s, near the 358GB/s per-core HBM cap).
  - The PE does ALL the reduction work (the DVE carries nothing
    during the stream): first w = Wl@Wr -> psum[1,512] and
    wc = dot(br,Wl) -> psum[1,1] via 4+4 lhsT=[128,1] matmuls off
    the packed weight tile, then xsum accumulates 16 matmuls
    (lhsT=ones[128,1], rhs=slice) into psum[1,512].
  - tail: one DVE scalar_tensor_tensor fuses xsum*w with the free-dim
    reduce into logit[1,1]; sigmoid with bias t2 = S*wc + bl
    (precomputed mid-stream on the scalar engine, table prewarmed);
    DMA the [1,1] out on the scalar ring.
"""

import numpy as np

B, S, D = 8, 2048, 512
P = 128
NSL = 16  # x slice DMAs per core (256KB each)
JW = 4  # Wr/Wl/br rows per partition
WBLK = D + 1  # per-j packed block: Wr row | br entry
WCOL = JW * WBLK + JW + 1  # 4 blocks | wl (4) | bl (1) = 2057
N_SYNC = 10  # x slices on the sync ring; rest follow wp on the scalar ring

_CACHE = {}


def _build():
    import concourse.bacc as bacc
    import concourse.mybir as mybir
    import concourse.tile as tile

    f32 = mybir.dt.float32

    nc = bacc.Bacc(
        "TRN2",
        target_bir_lowering=False,
        debug=False,
        enable_asserts=False,
        num_devices=B,
    )
    x_d = nc.dram_tensor("x", [NSL, P, D], f32, kind="ExternalInput").ap()
    wp_d = nc.dram_tensor("wp", [P, WCOL], f32, kind="ExternalInput").ap()
    out_d = nc.dram_tensor("out", [1, 1], f32, kind="ExternalOutput").ap()

    with tile.TileContext(nc) as tc:
        with (
            tc.tile_pool(name="xp", bufs=NSL) as xp,
            tc.tile_pool(name="sg", bufs=1) as sg,
            tc.tile_pool(name="ps", bufs=1, space="PSUM") as ps,
        ):
            # One DMA for every weight byte, first on the scalar ring.
            wp = sg.tile([P, WCOL], f32, tag="wp")
            nc.scalar.dma_start(wp, wp_d)

            # x slices: sync ring first (it has nothing else), then the
            # scalar ring behind the weights.
            xts = []
            for n in range(NSL):
                xt = xp.tile([P, D], f32, tag="xt")
                eng = nc.sync if n < N_SYNC else nc.scalar
                eng.dma_start(xt, x_d[n])
                xts.append(xt)

            ones = sg.tile([P, 1], f32, tag="ones")
            nc.vector.memset(ones, 1.0)
            # Prewarm the sigmoid activation table (1.3us) off the
            # critical path, after the scalar engine's dma_starts.
            warm = sg.tile([1, 1], f32, tag="warm")
            nc.scalar.activation(
                warm, ones[0:1, 0:1], mybir.ActivationFunctionType.Sigmoid
            )

            wlt = wp[:, JW * WBLK : JW * WBLK + JW]
            blt = wp[0:1, JW * WBLK + JW : JW * WBLK + JW + 1]

            # w = Wl@Wr as [1,512] and wc = dot(br,Wl) as [1,1], both on
            # the PE straight off the packed weight tile (contraction dim
            # = partitions; the j loop accumulates the 4 row groups).
            w_ps = ps.tile([1, D], f32, tag="w")
            for j in range(JW):
                nc.tensor.matmul(
                    w_ps,
                    wlt[:, j : j + 1],
                    wp[:, j * WBLK : j * WBLK + D],
                    start=(j == 0),
                    stop=(j == JW - 1),
                )
            wc_ps = ps.tile([1, 1], f32, tag="wc")
            for j in range(JW):
                nc.tensor.matmul(
                    wc_ps,
                    wlt[:, j : j + 1],
                    wp[:, j * WBLK + D : (j + 1) * WBLK],
                    start=(j == 0),
                    stop=(j == JW - 1),
                )

            # t2 = S * dot(br,Wl) + bl, mid-stream on the scalar engine.
            t2 = sg.tile([1, 1], f32, tag="t2")
            nc.scalar.activation(
                t2,
                wc_ps,
                mybir.ActivationFunctionType.Copy,
                scale=float(S),
            )
            nc.vector.tensor_add(out=t2, in0=t2, in1=blt)

            # w to SBUF mid-stream (the tail DVE op may read only one
            # PSUM operand, and xsum stays in PSUM until the end).
            w_sb = sg.tile([1, D], f32, tag="w_sb")
            nc.scalar.activation(w_sb, w_ps, mybir.ActivationFunctionType.Copy)

            # xsum[1,512] accumulates all 16 slices on the PE
            # (lhsT = ones column -> column sums into psum partition 0).
            xs_ps = ps.tile([1, D], f32, tag="xs")
            for n in range(NSL):
                nc.tensor.matmul(
                    xs_ps,
                    ones,
                    xts[n],
                    start=(n == 0),
                    stop=(n == NSL - 1),
                )

            # tail: logit = sum_d xsum[d]*w[d] fused in one DVE op.
            prod = sg.tile([1, D], f32, tag="prod")
            logit = sg.tile([1, 1], f32, tag="logit")
            nc.vector.scalar_tensor_tensor(
                out=prod,
                in0=xs_ps,
                scalar=1.0,
                in1=w_sb,
                op0=mybir.AluOpType.mult,
                op1=mybir.AluOpType.mult,
                accum_out=logit,
            )
            fin = sg.tile([1, 1], f32, tag="fin")
            nc.scalar.activation(
                fin,
                logit,
                mybir.ActivationFunctionType.Sigmoid,
                bias=t2,
                scale=1.0,
            )
            nc.scalar.dma_start(out_d, fin)

    nc.compile()
    return nc


def _in_maps(inputs):
    x = np.ascontiguousarray(np.asarray(inputs["x"], dtype=np.float32))
    Wr = np.asarray(inputs["Wr"], dtype=np.float32)
    br = np.asarray(inputs["br"], dtype=np.float32)
    Wl = np.asarray(inputs["Wl"], dtype=np.float32)
    bl = np.asarray(inputs["bl"], dtype=np.float32)

    wp = np.zeros((P, WCOL), dtype=np.float32)
    blk = wp[:, : JW * WBLK].reshape(P, JW, WBLK)
    blk[:, :, :D] = Wr.reshape(P, JW, D)
    blk[:, :, D] = br.reshape(P, JW)
    wp[:, JW * WBLK : JW * WBLK + JW] = Wl.reshape(P, JW)
    wp[0, JW * WBLK + JW] = bl[0]

    return [{"x": x[b].reshape(NSL, P, D), "wp": wp} for b in range(B)]


def get_nc():
    if "nc" not in _CACHE:
        _CACHE["nc"] = _build()
    return _CACHE["nc"]


def kernel(**inputs) -> np.ndarray:
    from concourse.bass_utils import run_bass_kernel_spmd

    nc = get_nc()
    res = run_bass_kernel_spmd(nc, _in_maps(inputs), list(range(B)))
    out = np.stack([res.results[b]["out"].reshape(()) for b in range(B)])
    return out.reshape(B, 1).astype(np.float32)


# revision 7
# speedup vs baseline: 1.0922x; 1.0922x over previous
"""Trainium2 Bass kernel for nn_LogLinearAttention.

Math: the reference computes
    q = x@Wq.T+bq ; v = x@Wv.T+bv ; r = x@Wr.T+br
    scores = q @ v.T ; attn = softmax(scores, axis=1)   # over the QUERY axis
    emb[b,s,:] = sum_t attn[b,s,t] r[b,t,:] ; pooled = emb.sum(axis=1)
    out = sigmoid(pooled @ Wl.T + bl)

Because softmax normalizes over axis 1 and pooled sums over that same
axis, sum_s attn[s, t] == 1 for every t, so
    pooled[b] = sum_t r[b, t, :] = (sum_t x[b, t, :]) @ Wr.T + S*br
and the q/v projections and the S x S attention cancel exactly:
    out[b] = sigmoid( xsum[b] . (Wl@Wr) + S*(br . Wl) + bl )

The kernel therefore only needs a sequence-sum of x (the only large
input, 32MB total) plus tiny weight contractions. Data-parallel over
batch: core b handles x[b] (4MB), weights replicated.

Per-core device program (v6):
  - v4 AllGathered a sharded Wl@Wr; the trace showed that 32KB
    collective costing ~65us of fixed latency. v5 replicated Wr and
    accumulated x on the PE; fp32 PE matmuls run at 1/4 rate and
    became a 19us critical path. v6:
  - x[b] arrives as 16 slice DMAs of [128,512] fp32 (256KB each),
    9 on the sync HWDGE ring, 7 behind the weights on the scalar
    ring (only SP+Activation have HWDGE; together they run at the
    ~400GB/s per-core cap, so bytes are the binding constraint).
  - ALL weights ship as ONE bf16 [128,2058] DMA (0.53MB instead of
    1.05MB fp32 - weight precision 0.4% << the 2e-2 tolerance):
    per j in 0..3 a [Wr rows 4p+j | br col] block of width 513,
    then Wl as [128,4], a unit column and bl/S (folds bl into the
    br.Wl contraction so no scalar fixup is needed).
  - DVE accumulates the x stream: one tensor_add per slice in
    arrival-interleaved order (690ns each vs ~745ns arrival cadence).
  - PE (bf16, full rate) computes w_rep[128,512] = broadcast(Wl@Wr)
    via free-dim-broadcast lhsT, and wc = dot(br,Wl)+bl/S, all
    mid-stream. t2 = S*wc on the scalar engine (sigmoid table
    prewarmed).
  - tail: ONE DVE scalar_tensor_tensor fuses acc*w_rep with the
    free-dim reduce -> red[128,1]; PE contracts red over partitions
    with a ones column -> logit[1,1]; sigmoid(logit + t2); DMA the
    [1,1] out on the scalar ring.
"""

import numpy as np

B, S, D = 8, 2048, 512
P = 128
NSL = 16  # x slice DMAs per core (256KB each)
JW = 4  # Wr/Wl/br rows per partition
WBLK = D + 1  # per-j packed block: Wr row | br entry
WCOL = JW * WBLK + JW + 2  # 4 blocks | wl (4) | e0, bl/S = 2058
N_SYNC = 9  # x slices on the sync ring; rest follow wp on the scalar ring

_CACHE = {}


def _build():
    import concourse.bacc as bacc
    import concourse.mybir as mybir
    import concourse.tile as tile

    f32 = mybir.dt.float32
    bf16 = mybir.dt.bfloat16

    nc = bacc.Bacc(
        "TRN2",
        target_bir_lowering=False,
        debug=False,
        enable_asserts=False,
        num_devices=B,
    )
    x_d = nc.dram_tensor("x", [NSL, P, D], f32, kind="ExternalInput").ap()
    wp_d = nc.dram_tensor("wp", [P, WCOL], bf16, kind="ExternalInput").ap()
    out_d = nc.dram_tensor("out", [1, 1], f32, kind="ExternalOutput").ap()

    with tile.TileContext(nc) as tc:
        with (
            tc.tile_pool(name="xp", bufs=NSL) as xp,
            tc.tile_pool(name="sg", bufs=1) as sg,
            tc.tile_pool(name="ps", bufs=1, space="PSUM") as ps,
        ):
            # One DMA for every weight byte, first on the scalar ring.
            wp = sg.tile([P, WCOL], bf16, tag="wp")
            nc.scalar.dma_start(wp, wp_d)

            # x slices: sync ring first (it has nothing else), then the
            # scalar ring behind the weights.
            xts = []
            for n in range(NSL):
                xt = xp.tile([P, D], f32, tag="xt")
                eng = nc.sync if n < N_SYNC else nc.scalar
                eng.dma_start(xt, x_d[n])
                xts.append(xt)

            ones = sg.tile([P, 1], f32, tag="ones")
            nc.vector.memset(ones, 1.0)
            # Prewarm the sigmoid activation table (1.3us) off the
            # critical path, after the scalar engine's dma_starts.
            warm = sg.tile([1, 1], f32, tag="warm")
            nc.scalar.activation(
                warm, ones[0:1, 0:1], mybir.ActivationFunctionType.Sigmoid
            )

            wlt = wp[:, JW * WBLK : JW * WBLK + JW]
            unit = wp[:, JW * WBLK + JW : JW * WBLK + JW + 1]
            blS = wp[:, JW * WBLK + JW + 1 : JW * WBLK + JW + 2]

            # w_rep[128,512] = Wl@Wr broadcast over partitions: lhsT is
            # the [128,1] Wl column j broadcast across the free dim, rhs
            # the Wr row block j; the j loop accumulates in PSUM.
            wrep_ps = ps.tile([P, D], f32, tag="wrep")
            for j in range(JW):
                nc.tensor.matmul(
                    wrep_ps,
                    wlt[:, j : j + 1].to_broadcast([P, P]),
                    wp[:, j * WBLK : j * WBLK + D],
                    start=(j == 0),
                    stop=(j == JW - 1),
                )
            # wc = dot(br,Wl) + bl/S (the packed unit column e0 carries
            # bl/S into the same accumulation).
            wc_ps = ps.tile([1, 1], f32, tag="wc")
            for j in range(JW):
                nc.tensor.matmul(
                    wc_ps,
                    wlt[:, j : j + 1],
                    wp[:, j * WBLK + D : (j + 1) * WBLK],
                    start=(j == 0),
                    stop=False,
                )
            nc.tensor.matmul(wc_ps, unit, blS, start=False, stop=True)

            # t2 = S * wc, mid-stream on the scalar engine.
            t2 = sg.tile([1, 1], f32, tag="t2")
            nc.scalar.activation(
                t2,
                wc_ps,
                mybir.ActivationFunctionType.Copy,
                scale=float(S),
            )

            # acc[128,512] accumulates the x stream on the vector engine
            # in ring-interleaved (arrival) order.
            order = []
            a, b = 0, N_SYNC
            take_sync = True
            while len(order) < NSL:
                if take_sync and a < N_SYNC:
                    order.append(a)
                    a += 1
                elif b < NSL:
                    order.append(b)
                    b += 1
                else:
                    order.append(a)
                    a += 1
                take_sync = not take_sync
            acc = sg.tile([P, D], f32, tag="acc")
            nc.vector.tensor_add(out=acc, in0=xts[order[0]], in1=xts[order[1]])
            for n in order[2:]:
                nc.vector.tensor_add(out=acc, in0=acc, in1=xts[n])

            # tail: red[p] = sum_d acc[p,d]*w_rep[p,d] fused in one DVE
            # op (w_rep may stay in PSUM: only one PSUM operand).
            prod = sg.tile([P, D], f32, tag="prod")
            red = sg.tile([P, 1], f32, tag="red")
            nc.vector.scalar_tensor_tensor(
                out=prod,
                in0=acc,
                scalar=1.0,
                in1=wrep_ps,
                op0=mybir.AluOpType.mult,
                op1=mybir.AluOpType.mult,
                accum_out=red,
            )
            logit_ps = ps.tile([1, 1], f32, tag="logit")
            nc.tensor.matmul(logit_ps, red, ones, start=True, stop=True)
            fin = sg.tile([1, 1], f32, tag="fin")
            nc.scalar.activation(
                fin,
                logit_ps,
                mybir.ActivationFunctionType.Sigmoid,
                bias=t2,
                scale=1.0,
            )
            nc.scalar.dma_start(out_d, fin)

    nc.compile()
    return nc


def _in_maps(inputs):
    x = np.ascontiguousarray(np.asarray(inputs["x"], dtype=np.float32))
    Wr = np.asarray(inputs["Wr"], dtype=np.float32)
    br = np.asarray(inputs["br"], dtype=np.float32)
    Wl = np.asarray(inputs["Wl"], dtype=np.float32)
    bl = np.asarray(inputs["bl"], dtype=np.float32)

    wp = np.zeros((P, WCOL), dtype=np.float32)
    blk = wp[:, : JW * WBLK].reshape(P, JW, WBLK)
    blk[:, :, :D] = Wr.reshape(P, JW, D)
    blk[:, :, D] = br.reshape(P, JW)
    wp[:, JW * WBLK : JW * WBLK + JW] = Wl.reshape(P, JW)
    wp[0, JW * WBLK + JW] = 1.0
    wp[0, JW * WBLK + JW + 1] = bl[0] / float(S)
    import ml_dtypes

    wp16 = wp.astype(ml_dtypes.bfloat16)

    return [{"x": x[b].reshape(NSL, P, D), "wp": wp16} for b in range(B)]


def get_nc():
    if "nc" not in _CACHE:
        _CACHE["nc"] = _build()
    return _CACHE["nc"]


def kernel(**inputs) -> np.ndarray:
    from concourse.bass_utils import run_bass_kernel_spmd

    nc = get_nc()
    res = run_bass_kernel_spmd(nc, _in_maps(inputs), list(range(B)))
    out = np.stack([res.results[b]["out"].reshape(()) for b in range(B)])
    return out.reshape(B, 1).astype(np.float32)
